# revision 12
# baseline (speedup 1.0000x reference)
"""NT-Xent (SimCLR) contrastive loss on 8 Trainium2 NeuronCores.

Math: with x = row-normalized representation [8192, 256], tau = 0.5,
  sim = x @ x.T
  loss = (1/8192) * sum_i [ ln(sum_{j != i} exp(2 sim[i,j])) - 2 sim[i, pos(i)] ]
where pos(i) = (i + 4096) mod 8192.

Split of work:
  Host (numpy): row-normalize, cast to bf16, per-core row-rolled copies,
  positive-pair dots, column sums of the exported exp blocks, final
  combine/ln/mean.
  Device (O(N^2*D)): the similarity matrix and exp(2 sim).

Symmetry: sim is symmetric, so only 5/8 of it is computed. Each core gets
x rolled so its 1024 slab rows sit at rows 0..1023 and computes
sim[0:1024, 0:5120] (column blocks b=0..4; b=4 is computed by both members
of a (c, c+4) pair). Row sums of exp(2 sim) cover blocks 0..4; the missing
blocks 5..7 are transposes of blocks 1..3 of three other cores, recovered
from COLUMN sums over blocks 1..3: the exp'd values for those columns are
DMA'd back to DRAM as bf16 and the host sums them (O(N^2) adds on host,
but off the device critical path).

Device kernel (SPMD, identical program on all 8 cores):
  1. 8 xbar transpose-DMAs (DRAM bf16 -> SBUF, split over the sync and
     scalar HWDGE rings) build xT tiles for columns 0..5120; the slab
     tile (cols 0..1024) and the last chunk first.
  2. Three chunk-sweeps in order c2 [4096:5120), c1 [2048:4096),
     c0 [0:2048): per slab row-tile m (8), k-outer bf16 matmuls (N=512)
     into a [128,2048] PSUM tile (2 bufs = all 8 banks), then one scalar
     Exp (scale=2) per chunk -> bf16 SBUF. Sweeping chunk-outer makes the
     first sweep depend on one small DMA and keeps the PE stream dense
     (HAM warm). The exp activation table is preloaded during the fill.
  3. Row sums: DVE reduce_sum for sweeps c2/c1, scalar accum_out for the
     final c0 sweep -> rs [128, 24]. Column data: c1 chunks and the
     [1024:2048) half of c0 chunks are DMA'd out (6 MB bf16, overlapped).
"""

import numpy as np
import ml_dtypes

import concourse.bacc as bacc
import concourse.tile as tile
from concourse import mybir
from concourse.bass_utils import run_bass_kernel_spmd

N2 = 8192            # total rows (2N)
D = 256              # feature dim
NCORES = 8
ROWS = N2 // NCORES  # 1024 slab rows per core
N = N2 // 2          # positive-pair offset
P = 128              # SBUF partitions
KC = D // P          # 2 contraction chunks of 128
MT = ROWS // P       # 8 slab row-tiles
COLS = 5120          # columns computed per slab row-tile (blocks 0..4)
# xT tiles per k-half: (start, width); A holds the slab columns
XT = [(0, 1024), (1024, 1024), (2048, 2048), (4096, 1024)]
# chunk sweeps in execution order: (col start, width, exp chunk id)
SWEEPS = [(4096, 1024, 2), (2048, 2048, 1), (0, 2048, 0)]
NCH = 3
DVE_CH = (2, 1)      # chunks whose row sum goes to DVE reduce
EXW = 3072           # exported columns per row-tile (rolled cols 1024..4096)

F32 = mybir.dt.float32
BF16 = mybir.dt.bfloat16
AF = mybir.ActivationFunctionType


def _xt_tile(col):
    for t, (s, w) in enumerate(XT):
        if s <= col < s + w:
            return t, col - s
    raise ValueError(col)


def _build_kernel(tc: tile.TileContext, rs_out, eout, xk):
    nc = tc.nc
    with (
        tc.tile_pool(name="xt", bufs=1) as xtp,
        tc.tile_pool(name="acc", bufs=1) as accp,
        tc.tile_pool(name="esc0", bufs=4) as escp0,
        tc.tile_pool(name="esc1", bufs=6) as escp1,
        tc.tile_pool(name="esc2", bufs=4) as escp2,
        tc.tile_pool(name="ps", bufs=2, space="PSUM") as psp,
    ):
        xts = [[xtp.tile([P, w], BF16, tag=f"xt{k}_{t}", name=f"xt{k}_{t}")
                for t, (s, w) in enumerate(XT)] for k in range(KC)]
        rs = accp.tile([P, NCH * MT], F32, tag="rs", name="rs")
        ones = accp.tile([P, 1], BF16, tag="ones", name="ones")
        dume = accp.tile([P, 1], BF16, tag="dume", name="dume")
        nc.vector.memset(ones, 1.0)
        # pull the exp ACT_TABLE_LOAD into the DMA fill window
        nc.scalar.activation(dume, ones, AF.Exp, scale=1.0)

        # transposed bf16 keys; slab tile A and the first sweep's tile D
        # first, k=0 on the sync ring and k=1 on the scalar ring
        for t in (0, 3, 2, 1):
            s, w = XT[t]
            nc.sync.dma_start(out=xts[0][t], in_=xk[0][s:s + w, :],
                              transpose=True)
            nc.scalar.dma_start(out=xts[1][t], in_=xk[1][s:s + w, :],
                                transpose=True)

        pools = {0: escp0, 1: escp1, 2: escp2}
        for cs, w, c in SWEEPS:
            for m in range(MT):
                ps = psp.tile([P, 2048], F32, tag="ps", name="ps")
                for k in range(KC):  # k outer: stationary switches 2x/chunk
                    for half in range(w // 512):
                        t, off = _xt_tile(cs + half * 512)
                        nc.tensor.matmul(
                            ps[:, half * 512:(half + 1) * 512],
                            xts[k][0][:, m * P:(m + 1) * P],
                            xts[k][t][:, off:off + 512],
                            start=(k == 0), stop=(k == KC - 1))
                esc = pools[c].tile([P, w], BF16, tag="esc", name="esc")
                ridx = m * NCH + c
                if c in DVE_CH:  # row sum on DVE, off the scalar engine
                    nc.scalar.activation(esc, ps[:, :w], AF.Exp, scale=2.0)
                    nc.vector.reduce_sum(rs[:, ridx:ridx + 1], esc,
                                         axis=mybir.AxisListType.X)
                else:
                    nc.scalar.activation(
                        esc, ps[:, :w], AF.Exp, scale=2.0,
                        accum_out=rs[:, ridx:ridx + 1])
                # export exp'd columns in rolled range [1024, 4096) for
                # host-side column sums
                if c == 1:
                    nc.sync.dma_start(
                        out=eout[m * P:(m + 1) * P, 1024:1024 + 2048],
                        in_=esc)
                elif c == 0:
                    nc.sync.dma_start(
                        out=eout[m * P:(m + 1) * P, 0:1024],
                        in_=esc[:, 1024:2048])

        nc.sync.dma_start(out=rs_out, in_=rs)


def build_nc():
    nc = bacc.Bacc("TRN2", target_bir_lowering=False, debug=False,
                   num_devices=NCORES)
    xk = [nc.dram_tensor(f"x{k}", [N2, P], BF16, kind="ExternalInput").ap()
          for k in range(KC)]
    rs_out = nc.dram_tensor("rs", [P, NCH * MT], F32,
                            kind="ExternalOutput").ap()
    eout = nc.dram_tensor("eout", [ROWS, EXW], BF16,
                          kind="ExternalOutput").ap()
    with tile.TileContext(nc) as tc:
        _build_kernel(tc, rs_out, eout, xk)
    nc.compile()
    return nc


_NC = None
LAST_RESULTS = None


def _make_in_maps(xb16: np.ndarray):
    in_maps = []
    for c in range(NCORES):
        xr = np.roll(xb16, -c * ROWS, axis=0)
        in_maps.append({f"x{k}": np.ascontiguousarray(xr[:, k * P:(k + 1) * P])
                        for k in range(KC)})
    return in_maps


def kernel(representation: np.ndarray, **run_kwargs) -> np.ndarray:
    global _NC, LAST_RESULTS
    rep = np.asarray(representation, dtype=np.float32)
    assert rep.shape == (N2, D)

    # host prep: normalize (f32, matching torch CosineSimilarity eps), bf16
    norms = np.maximum(np.sqrt((rep.astype(np.float64) ** 2).sum(axis=1)),
                       1e-8)
    xn = (rep / norms[:, None]).astype(np.float32)
    xb16 = xn.astype(ml_dtypes.bfloat16)

    if _NC is None:
        _NC = build_nc()
    res = run_bass_kernel_spmd(_NC, _make_in_maps(xb16),
                               core_ids=list(range(NCORES)), **run_kwargs)
    LAST_RESULTS = res

    # combine row partials (rolled cols 0..5120) and host column sums of
    # the exported exp values (rolled cols 1024..4096) into full row sums
    S = np.zeros(N2, dtype=np.float64)
    for c, r in enumerate(res.results):
        rs = r["rs"].astype(np.float64).reshape(P, MT, NCH)  # [p, m, ch]
        own = rs.sum(axis=2).T.reshape(ROWS)                 # row m*128+p
        S[c * ROWS:(c + 1) * ROWS] += own
        colsum = r["eout"].astype(np.float64).sum(axis=0)    # [3072]
        for i in range(EXW // 512):
            g0 = (c * ROWS + 1024 + 512 * i) % N2
            S[g0:g0 + 512] += colsum[512 * i:512 * (i + 1)]

    # host tail: remove diagonal (as the device computed it, from bf16
    # inputs), add positive terms, final log/mean
    xb = xb16.astype(np.float64)
    ssb = (xb * xb).sum(axis=1)                  # device's sim[i,i]
    denom = S - np.exp(2.0 * ssb)
    xn64 = xn.astype(np.float64)
    pos = (xn64 * np.roll(xn64, -N, axis=0)).sum(axis=1)
    loss = (np.log(denom) - 2.0 * pos).mean()
    return np.asarray(np.float32(loss))


# revision 14
# speedup vs baseline: 1.0741x; 1.0741x over previous
"""NT-Xent (SimCLR) contrastive loss on 8 Trainium2 NeuronCores.

Math: with x = row-normalized representation [8192, 256], tau = 0.5,
  sim = x @ x.T
  loss = (1/8192) * sum_i [ ln(sum_{j != i} exp(2 sim[i,j])) - 2 sim[i, pos(i)] ]
where pos(i) = (i + 4096) mod 8192.

Split of work:
  Host (O(N*D), numpy): row-normalize, cast to bf16, per-core row-rolled
  and transposed copies, positive-pair dots, final combine/ln/mean.
  Device (O(N^2*D)): the similarity matrix and row/column sums of exp(2 sim).

Symmetry: sim is symmetric, so only 5/8 of it is computed. Each core gets
xT rolled so its 1024 slab rows sit at columns 0..1023 of xT, and computes
sim[0:1024, 0:5120] (column blocks b=0..4; b=4 is computed by both members
of a (c, c+4) pair). Row sums of exp(2 sim) cover blocks 0..4; the missing
blocks 5..7 are transposes of blocks 1..3 of three other cores, recovered
from COLUMN sums of exp over blocks 1..3, computed on-device with
ones-stationary matmuls accumulating in two PSUM banks across the 8
row-tiles. The host combines row + column partials into full row sums.

Device kernel (SPMD, identical program on all 8 cores):
  1. The host supplies xT [256, 8192] bf16 (normalized, rolled,
     transposed), so SBUF keys load with 5 plain chunk-aligned DMAs
     (both k-halves per chunk in one 3D-AP DMA). The slab tile and the
     first sweep's tile come first.
  2. Four chunk-sweeps over columns, widths {512,1536,1536,1536}, the
     cheap 512 sweep first (it depends on two small DMAs and its dense
     matmul stream warms the HAM clock gate). Per row-tile m: k-outer
     bf16 matmuls (N=512) into a [128,1536] PSUM tile (2 bufs, 6 banks),
     then one scalar Exp (scale=2) per chunk -> bf16 SBUF. The exp
     activation table is preloaded during the DMA fill.
  3. Row sums: DVE reduce_sum for the three 1536 sweeps (the DVE is
     otherwise idle), accum_out on the 512 sweep -> rs [128, 32].
  4. Column sums: six ones-stationary matmuls per m over exp'd
     [128,512] slices of blocks 1..3, accumulated over m in two PSUM
     banks at partitions {0,32,64}, emitted one m behind in the last
     sweep to keep the PE stream dense. DVE copies them out at the end.
"""

import numpy as np
import ml_dtypes

import concourse.bacc as bacc
import concourse.tile as tile
from concourse import mybir
from concourse.bass_utils import run_bass_kernel_spmd

N2 = 8192            # total rows (2N)
D = 256              # feature dim
NCORES = 8
ROWS = N2 // NCORES  # 1024 slab rows per core
N = N2 // 2          # positive-pair offset
P = 128              # SBUF partitions
KC = D // P          # 2 contraction chunks of 128
MT = ROWS // P       # 8 slab row-tiles
COLS = 5120          # columns computed per slab row-tile (blocks 0..4)
# xT SBUF tiles: (col start, width); tile 0 holds the slab columns
XT = [(0, 1024), (4608, 512), (1024, 512), (1536, 1536), (3072, 1536)]
# chunk sweeps in execution order: (col start, width, chunk id)
SWEEPS = [(4608, 512, 3), (0, 1536, 0), (1536, 1536, 1), (3072, 1536, 2)]
NCH = 4
ACC_CH = 3           # the only chunk using scalar accum_out (rest: DVE)
# column-sum slices covering rolled cols [1024, 4096) as (chunk, offset)
RED = [(0, 1024), (1, 0), (1, 512), (1, 1024), (2, 0), (2, 512)]

F32 = mybir.dt.float32
BF16 = mybir.dt.bfloat16
AF = mybir.ActivationFunctionType


def _xt_tile(col):
    for t, (s, w) in enumerate(XT):
        if s <= col < s + w:
            return t, col - s
    raise ValueError(col)


def _build_kernel(tc: tile.TileContext, rs_out, cols_out, xth):
    nc = tc.nc
    with (
        tc.tile_pool(name="xt", bufs=1) as xtp,
        tc.tile_pool(name="acc", bufs=1) as accp,
        tc.tile_pool(name="esc0", bufs=MT) as escp0,
        tc.tile_pool(name="esc1", bufs=MT) as escp1,
        tc.tile_pool(name="esc2", bufs=MT) as escp2,
        tc.tile_pool(name="esc3", bufs=2) as escp3,
        tc.tile_pool(name="ps", bufs=2, space="PSUM") as psp,
        tc.tile_pool(name="red", bufs=1, space="PSUM") as redp,
    ):
        # each xT tile holds both k-halves: [:, k*w : k*w+w]
        xts = [xtp.tile([P, KC * w], BF16, tag=f"xt{t}", name=f"xt{t}")
               for t, (s, w) in enumerate(XT)]
        rs = accp.tile([P, NCH * MT], F32, tag="rs", name="rs")
        ones = accp.tile([P, 1], BF16, tag="ones", name="ones")
        dume = accp.tile([P, 1], BF16, tag="dume", name="dume")
        colsb = accp.tile([P, 1024], F32, tag="colsb", name="colsb")
        red = [redp.tile([P, 512], F32, tag=f"red{t}", name=f"red{t}")
               for t in range(2)]
        nc.vector.memset(ones, 1.0)
        # pull the exp ACT_TABLE_LOAD into the DMA fill window
        nc.scalar.activation(dume, ones, AF.Exp, scale=1.0)

        # plain DMAs of host-pretransposed keys; one 3D-AP DMA per tile
        # loads both k-halves (dest free block k*w..k*w+w <- xT rows
        # k*128..k*128+128, cols s..s+w)
        for t, (s, w) in enumerate(XT):
            nc.sync.dma_start(
                out=xts[t],
                in_=xth[:, s:s + w].rearrange("(k p) c -> p k c", k=KC))

        def mov(k, col):  # moving operand [128, 512] for global column col
            t, off = _xt_tile(col)
            w = XT[t][1]
            return xts[t][:, k * w + off:k * w + off + 512]

        escs = {}
        pools = {0: escp0, 1: escp1, 2: escp2, 3: escp3}

        def red_mms(m):
            for i, (rc, off) in enumerate(RED):
                t, bp = i % 2, 32 * (i // 2)
                nc.tensor.matmul(
                    red[t][bp:bp + 1, :],
                    ones,
                    escs[(m, rc)][:, off:off + 512],
                    start=(m == 0), stop=(m == MT - 1),
                    skip_group_check=True)

        last = SWEEPS[-1][2]
        for cs, w, c in SWEEPS:
            for m in range(MT):
                ps = psp.tile([P, 1536], F32, tag="ps", name="ps")
                for k in range(KC):  # k outer: stationary switches 2x/chunk
                    for half in range(w // 512):
                        nc.tensor.matmul(
                            ps[:, half * 512:(half + 1) * 512],
                            xts[0][:, k * 1024 + m * P:k * 1024 + (m + 1) * P],
                            mov(k, cs + half * 512),
                            start=(k == 0), stop=(k == KC - 1))
                esc = pools[c].tile([P, w], BF16, tag="esc", name="esc")
                escs[(m, c)] = esc
                ridx = m * NCH + c
                if c == ACC_CH:
                    nc.scalar.activation(
                        esc, ps[:, :w], AF.Exp, scale=2.0,
                        accum_out=rs[:, ridx:ridx + 1])
                else:  # row sum on DVE, off the scalar engine
                    nc.scalar.activation(esc, ps[:, :w], AF.Exp, scale=2.0)
                    nc.vector.reduce_sum(rs[:, ridx:ridx + 1], esc,
                                         axis=mybir.AxisListType.X)
                # column-sum matmuls ride the last sweep, one m behind
                if c == last and m > 0:
                    red_mms(m - 1)
        red_mms(MT - 1)

        for t in range(2):
            nc.vector.tensor_copy(colsb[:, t * 512:(t + 1) * 512], red[t])
        nc.sync.dma_start(out=rs_out, in_=rs)
        nc.sync.dma_start(out=cols_out, in_=colsb)


def build_nc():
    nc = bacc.Bacc("TRN2", target_bir_lowering=False, debug=False,
                   num_devices=NCORES)
    xth = nc.dram_tensor("xt", [D, N2], BF16, kind="ExternalInput").ap()
    rs_out = nc.dram_tensor("rs", [P, NCH * MT], F32,
                            kind="ExternalOutput").ap()
    cols_out = nc.dram_tensor("cols", [P, 1024], F32,
                              kind="ExternalOutput").ap()
    with tile.TileContext(nc) as tc:
        _build_kernel(tc, rs_out, cols_out, xth)
    nc.compile()
    return nc


_NC = None
LAST_RESULTS = None


def _make_in_maps(xb16: np.ndarray):
    in_maps = []
    for c in range(NCORES):
        xr = np.roll(xb16, -c * ROWS, axis=0)
        in_maps.append({"xt": np.ascontiguousarray(xr.T)})
    return in_maps


def kernel(representation: np.ndarray, **run_kwargs) -> np.ndarray:
    global _NC, LAST_RESULTS
    rep = np.asarray(representation, dtype=np.float32)
    assert rep.shape == (N2, D)

    # host prep: normalize (f32, matching torch CosineSimilarity eps), bf16
    norms = np.maximum(np.sqrt((rep.astype(np.float64) ** 2).sum(axis=1)),
                       1e-8)
    xn = (rep / norms[:, None]).astype(np.float32)
    xb16 = xn.astype(ml_dtypes.bfloat16)

    if _NC is None:
        _NC = build_nc()
    res = run_bass_kernel_spmd(_NC, _make_in_maps(xb16),
                               core_ids=list(range(NCORES)), **run_kwargs)
    LAST_RESULTS = res

    # combine row partials (rolled cols 0..5120) and column partials
    # (rolled cols 1024..4096, blocks b=1..3) into full row sums S
    S = np.zeros(N2, dtype=np.float64)
    for c, r in enumerate(res.results):
        rs = r["rs"].astype(np.float64).reshape(P, MT, NCH)  # [p, m, ch]
        own = rs.sum(axis=2).T.reshape(ROWS)                 # row m*128+p
        S[c * ROWS:(c + 1) * ROWS] += own
        cols = r["cols"].astype(np.float64)                  # [p, 1024]
        for i in range(6):
            colsum = cols[32 * (i // 2), (i % 2) * 512:(i % 2) * 512 + 512]
            g0 = (c * ROWS + 1024 + 512 * i) % N2
            S[g0:g0 + 512] += colsum

    # host tail: remove diagonal (as the device computed it, from bf16
    # inputs), add positive terms, final log/mean
    xb = xb16.astype(np.float64)
    ssb = (xb * xb).sum(axis=1)                  # device's sim[i,i]
    denom = S - np.exp(2.0 * ssb)
    xn64 = xn.astype(np.float64)
    pos = (xn64 * np.roll(xn64, -N, axis=0)).sum(axis=1)
    loss = (np.log(denom) - 2.0 * pos).mean()
    return np.asarray(np.float32(loss))


# revision 22
# speedup vs baseline: 1.1209x; 1.0436x over previous
"""NT-Xent (SimCLR) contrastive loss on 8 Trainium2 NeuronCores.

Math: with x = row-normalized representation [8192, 256], tau = 0.5,
  sim = x @ x.T
  loss = (1/8192) * sum_i [ ln(sum_{j != i} exp(2 sim[i,j])) - 2 sim[i, pos(i)] ]
where pos(i) = (i + 4096) mod 8192.

Split of work:
  Host (O(N*D), numpy): row-normalize, cast to bf16, per-core row-rolled
  and transposed copies, positive-pair dots, final combine/ln/mean.
  Device (O(N^2*D)): the similarity matrix and row/column sums of exp(2 sim).

Symmetry: sim is symmetric, so only 5/8 of it is computed. Each core gets
xT rolled so its 1024 slab rows sit at columns 0..1023 of xT, and computes
sim[0:1024, 0:5120] (column blocks b=0..4; b=4 is computed by both members
of a (c, c+4) pair). Row sums of exp(2 sim) cover blocks 0..4; the missing
blocks 5..7 are transposes of blocks 1..3 of three other cores, recovered
from COLUMN sums of exp over blocks 1..3, computed on-device with
ones-stationary matmuls accumulating in two PSUM banks across the 8
row-tiles. The host combines row + column partials into full row sums.

Device kernel (SPMD, identical program on all 8 cores):
  1. The host supplies xT [256, 8192] bf16 (normalized, rolled,
     transposed), so SBUF keys load with 5 plain chunk-aligned DMAs
     (both k-halves per chunk in one 3D-AP DMA). The slab tile and the
     first sweep's tile come first.
  2. Four chunk-sweeps over columns, widths {512,1536,1536,1536}, the
     cheap 512 sweep first (it depends on two small DMAs and its dense
     matmul stream warms the HAM clock gate). Per row-tile m: k-outer
     bf16 matmuls (N=512) into a [128,1536] PSUM tile (2 bufs, 6 banks),
     then one scalar Exp (scale=2) per chunk -> bf16 SBUF. The exp
     activation table is preloaded during the DMA fill.
  3. Row sums: DVE reduce_sum for the three 1536 sweeps (the DVE is
     otherwise idle), accum_out on the 512 sweep -> rs [128, 32].
  4. Column sums: six ones-stationary matmuls per m over exp'd
     [128,512] slices of blocks 1..3, accumulated over m in two PSUM
     banks at partitions {0,32,64}, emitted one m behind in the last
     sweep to keep the PE stream dense. DVE copies them out at the end.
"""

import numpy as np
import ml_dtypes

import concourse.bacc as bacc
import concourse.tile as tile
from concourse import mybir
from concourse.bass_utils import run_bass_kernel_spmd

N2 = 8192            # total rows (2N)
D = 256              # feature dim
NCORES = 8
ROWS = N2 // NCORES  # 1024 slab rows per core
N = N2 // 2          # positive-pair offset
P = 128              # SBUF partitions
KC = D // P          # 2 contraction chunks of 128
MT = ROWS // P       # 8 slab row-tiles
COLS = 5120          # columns computed per slab row-tile (blocks 0..4)
# xT SBUF tiles: (col start, width); tile 0 holds the slab columns
XT = [(0, 1024), (4608, 512), (1024, 512), (1536, 1536), (3072, 1536)]
# chunk sweeps in execution order: (col start, width, chunk id)
SWEEPS = [(4608, 512, 3), (0, 1536, 0), (1536, 1536, 1), (3072, 1536, 2)]
NCH = 4
ACC_CH = 3           # the only chunk using scalar accum_out (rest: DVE)
# column-sum slices covering rolled cols [1024, 4096) as (chunk, offset)
RED = [(0, 1024), (1, 0), (1, 512), (1, 1024), (2, 0), (2, 512)]

F32 = mybir.dt.float32
BF16 = mybir.dt.bfloat16
F8 = mybir.dt.float8e4
AF = mybir.ActivationFunctionType
QS = 16.0            # fp8 quantization scale for xT (values ~N(0, 1/16))
ESC = 2.0 / (QS * QS)  # exp scale: sim = psum / QS^2, arg = 2*sim


def _xt_tile(col):
    for t, (s, w) in enumerate(XT):
        if s <= col < s + w:
            return t, col - s
    raise ValueError(col)


def _build_kernel(tc: tile.TileContext, rs_out, cols_out, xth):
    nc = tc.nc
    with (
        tc.tile_pool(name="xt", bufs=1) as xtp,
        tc.tile_pool(name="acc", bufs=1) as accp,
        tc.tile_pool(name="esc0", bufs=MT) as escp0,
        tc.tile_pool(name="esc1", bufs=MT) as escp1,
        tc.tile_pool(name="esc2", bufs=MT) as escp2,
        tc.tile_pool(name="esc3", bufs=2) as escp3,
        tc.tile_pool(name="ps", bufs=2, space="PSUM") as psp,
        tc.tile_pool(name="red", bufs=1, space="PSUM") as redp,
    ):
        # each xT tile holds both k-halves: [:, k*w : k*w+w]
        xts = [xtp.tile([P, KC * w], F8, tag=f"xt{t}", name=f"xt{t}")
               for t, (s, w) in enumerate(XT)]
        rs = accp.tile([P, NCH * MT], F32, tag="rs", name="rs")
        ones = accp.tile([P, 1], BF16, tag="ones", name="ones")
        dume = accp.tile([P, 1], BF16, tag="dume", name="dume")
        colsb = accp.tile([P, 1024], F32, tag="colsb", name="colsb")
        red = [redp.tile([P, 512], F32, tag=f"red{t}", name=f"red{t}")
               for t in range(2)]
        nc.vector.memset(ones, 1.0)
        # pull the exp ACT_TABLE_LOAD into the DMA fill window
        nc.scalar.activation(dume, ones, AF.Exp, scale=1.0)

        # plain DMAs of host-pretransposed keys; one 3D-AP DMA per tile
        # loads both k-halves (dest free block k*w..k*w+w <- xT rows
        # k*128..k*128+128, cols s..s+w)
        for t, (s, w) in enumerate(XT):
            nc.sync.dma_start(
                out=xts[t],
                in_=xth[:, s:s + w].rearrange("(k p) c -> p k c", k=KC))

        # 3D views [p, k, c] for DoubleRow matmuls (both k-tiles in one MM)
        xt3 = [xts[t].rearrange("p (k c) -> p k c", k=KC)
               for t in range(len(XT))]

        def mov(col):  # moving operand [128, 2, 512] for global column col
            t, off = _xt_tile(col)
            return xt3[t][:, :, off:off + 512]

        escs = {}
        pools = {0: escp0, 1: escp1, 2: escp2, 3: escp3}

        def red_mms(m):
            for i, (rc, off) in enumerate(RED):
                t, bp = i % 2, 32 * (i // 2)
                nc.tensor.matmul(
                    red[t][bp:bp + 1, :],
                    ones,
                    escs[(m, rc)][:, off:off + 512],
                    start=(m == 0), stop=(m == MT - 1),
                    skip_group_check=True)

        last = SWEEPS[-1][2]
        for cs, w, c in SWEEPS:
            for m in range(MT):
                ps = psp.tile([P, 1536], F32, tag="ps", name="ps")
                for half in range(w // 512):
                    nc.tensor.matmul(
                        ps[:, half * 512:(half + 1) * 512],
                        xt3[0][:, :, m * P:(m + 1) * P],
                        mov(cs + half * 512),
                        start=True, stop=True,
                        perf_mode=mybir.MatmulPerfMode.DoubleRow)
                esc = pools[c].tile([P, w], BF16, tag="esc", name="esc")
                escs[(m, c)] = esc
                ridx = m * NCH + c
                if c == ACC_CH:
                    nc.scalar.activation(
                        esc, ps[:, :w], AF.Exp, scale=ESC,
                        accum_out=rs[:, ridx:ridx + 1])
                else:  # row sum on DVE, off the scalar engine
                    nc.scalar.activation(esc, ps[:, :w], AF.Exp, scale=ESC)
                    nc.vector.reduce_sum(rs[:, ridx:ridx + 1], esc,
                                         axis=mybir.AxisListType.X)
                # column-sum matmuls ride the last sweep, one m behind
                if c == last and m > 0:
                    red_mms(m - 1)
        red_mms(MT - 1)

        for t in range(2):
            nc.vector.tensor_copy(colsb[:, t * 512:(t + 1) * 512], red[t])
        nc.sync.dma_start(out=rs_out, in_=rs)
        nc.sync.dma_start(out=cols_out, in_=colsb)


def build_nc():
    nc = bacc.Bacc("TRN2", target_bir_lowering=False, debug=False,
                   num_devices=NCORES)
    xth = nc.dram_tensor("xt", [D, N2], F8, kind="ExternalInput").ap()
    rs_out = nc.dram_tensor("rs", [P, NCH * MT], F32,
                            kind="ExternalOutput").ap()
    cols_out = nc.dram_tensor("cols", [P, 1024], F32,
                              kind="ExternalOutput").ap()
    with tile.TileContext(nc) as tc:
        _build_kernel(tc, rs_out, cols_out, xth)
    nc.compile()
    return nc


_NC = None
LAST_RESULTS = None


def _make_in_maps(xq: np.ndarray):
    in_maps = []
    for c in range(NCORES):
        xr = np.roll(xq, -c * ROWS, axis=0)
        in_maps.append({"xt": np.ascontiguousarray(xr.T)})
    return in_maps


def kernel(representation: np.ndarray, **run_kwargs) -> np.ndarray:
    global _NC, LAST_RESULTS
    rep = np.asarray(representation, dtype=np.float32)
    assert rep.shape == (N2, D)

    # host prep: normalize (f32, matching torch CosineSimilarity eps),
    # scale by QS and quantize to fp8e4m3 for DoubleRow matmuls
    norms = np.maximum(np.sqrt((rep.astype(np.float64) ** 2).sum(axis=1)),
                       1e-8)
    xn = (rep / norms[:, None]).astype(np.float32)
    xq = (xn * QS).astype(ml_dtypes.float8_e4m3fn)

    if _NC is None:
        _NC = build_nc()
    res = run_bass_kernel_spmd(_NC, _make_in_maps(xq),
                               core_ids=list(range(NCORES)), **run_kwargs)
    LAST_RESULTS = res

    # combine row partials (rolled cols 0..5120) and column partials
    # (rolled cols 1024..4096, blocks b=1..3) into full row sums S
    S = np.zeros(N2, dtype=np.float64)
    for c, r in enumerate(res.results):
        rs = r["rs"].astype(np.float64).reshape(P, MT, NCH)  # [p, m, ch]
        own = rs.sum(axis=2).T.reshape(ROWS)                 # row m*128+p
        S[c * ROWS:(c + 1) * ROWS] += own
        cols = r["cols"].astype(np.float64)                  # [p, 1024]
        for i in range(6):
            colsum = cols[32 * (i // 2), (i % 2) * 512:(i % 2) * 512 + 512]
            g0 = (c * ROWS + 1024 + 512 * i) % N2
            S[g0:g0 + 512] += colsum

    # host tail: remove diagonal (as the device computed it, from fp8
    # inputs), add positive terms, final log/mean
    xb = xq.astype(np.float64) / QS
    ssb = (xb * xb).sum(axis=1)                  # device's sim[i,i]
    denom = S - np.exp(2.0 * ssb)
    xn64 = xn.astype(np.float64)
    pos = (xn64 * np.roll(xn64, -N, axis=0)).sum(axis=1)
    loss = (np.log(denom) - 2.0 * pos).mean()
    return np.asarray(np.float32(loss))
